# revision 11
# baseline (speedup 1.0000x reference)
"""Bass/Trainium2 kernel for nn_MetaLoss_32255204393276.

Loss = BCE(sigmoid(pred), t) + boundary(EDT) + focal + SSIM over 8 pairs of
[1,192,192] masks; data-parallel over 8 NeuronCores (one image per core).
Each core returns [128, NCOL] partial sums; the host combines them.

EDT strategy: exact Euclidean distance transform via two min-plus passes
(rows then columns). For small window radius R the parabola (j-R)^2 is
symmetric, so the windowed min-plus is computed as R pair-mins of shifted
slices plus a fused (add-const, min) chain - exact since all finite values
are small integers (representable in bf16). Windowed results equal the true
EDT whenever max d^2 < (R+1)^2, certified on device; on failure (never for
the Bernoulli masks this problem generates) a full-window R=191 kernel
reruns, which is unconditionally exact including the empty-mask clamp.
"""

import numpy as np
from contextlib import ExitStack

import concourse.bass as bass
import concourse.bacc as bacc
import concourse.hw_specs as hw_specs
import concourse.tile as tile
from concourse import mybir
from concourse.bass_utils import run_bass_kernel_spmd

F32 = mybir.dt.float32
BF16 = mybir.dt.bfloat16
AF = mybir.ActivationFunctionType
OP = mybir.AluOpType
AX = mybir.AxisListType

H = W = 192
CONV = 182          # 192 - 11 + 1
BIG = 1e10
MAXD2 = float((H - 1) ** 2 + (W - 1) ** 2)  # 72962
EPS = 1e-6

# output columns
C_LNS0, C_LNS1 = 0, 1        # sum ln(1+e^-x)
C_SXM0, C_SXM1 = 2, 3        # sum x*(t-1)
C_FOC0, C_FOC1 = 4, 5        # sum alpha_t*(1-pt)^2*ln(pt)
C_BND = (6, 7, 8, 9)         # sum p*f(d2) per (F0,B0,F1,B1)
C_CERT = (10, 11, 12, 13)    # max clamped d2 per tile
C_SSA, C_SSB = 14, 15        # ssim ratio sums
NCOL = 16

_orig_get_tables = hw_specs.get_activation_tables


def _one_set_tables(module_arch):
    # all activation funcs we use (exp/ln/copy/abs/identity) live in this one
    # set; restricting the choice stops the compiler from thrashing tables
    tabs = _orig_get_tables(module_arch)
    name = "natural_log_exp_and_others"
    return {k: (v if k == name else set()) for k, v in tabs.items()}


def build_kernel(R=3, XB=192):
    K = 2 * R + 1
    PADF = W + 2 * R
    small = R <= 8
    EDT_DT = BF16 if small else F32
    EBIG = 512.0 if small else BIG

    bacc.get_activation_tables = _one_set_tables
    nc = bacc.Bacc(None, target_bir_lowering=False)
    pred = nc.dram_tensor("pred", [H, W], F32, kind="ExternalInput")
    targ = nc.dram_tensor("targ", [H, W], F32, kind="ExternalInput")
    if not small:
        qwin = nc.dram_tensor("qwin", [128, K], F32, kind="ExternalInput")
    gmat = nc.dram_tensor("gmat", [H, CONV], F32, kind="ExternalInput")
    ident = nc.dram_tensor("ident", [128, 128], F32, kind="ExternalInput")
    identb = nc.dram_tensor("identb", [128, 128], BF16, kind="ExternalInput")
    outd = nc.dram_tensor("out", [128, NCOL], F32, kind="ExternalOutput")

    with tile.TileContext(nc) as tc, ExitStack() as ctx:
        pool = ctx.enter_context(tc.tile_pool(name="main", bufs=1))
        psp = ctx.enter_context(tc.tile_pool(name="ps", bufs=3, space="PSUM"))
        psc = ctx.enter_context(tc.tile_pool(name="psc", bufs=1, space="PSUM"))

        OUT = pool.tile([128, NCOL], F32)
        nc.vector.memset(OUT[:], 0.0)

        # ---- loads ----
        P0 = pool.tile([128, W], F32)
        nc.sync.dma_start(P0[:], pred[0:128, :])
        P1 = pool.tile([128, W], F32)
        nc.sync.dma_start(P1[:64, :], pred[128:192, :])
        T0 = pool.tile([128, W], F32)
        nc.sync.dma_start(T0[:], targ[0:128, :])
        T1 = pool.tile([128, W], F32)
        nc.sync.dma_start(T1[:64, :], targ[128:192, :])
        if not small:
            qt = pool.tile([128, K], F32)
            nc.gpsimd.dma_start(qt[:], qwin[:])
        g0 = pool.tile([128, CONV], F32)
        nc.gpsimd.dma_start(g0[:], gmat[0:128, :])
        g1 = pool.tile([128, CONV], F32)
        nc.gpsimd.dma_start(g1[:64, :], gmat[128:192, :])
        idt = pool.tile([128, 128], F32)
        nc.gpsimd.dma_start(idt[:], ident[:])
        idtb = pool.tile([128, 128], BF16)
        nc.gpsimd.dma_start(idtb[:], identb[:])

        parts = [(P0, T0, 128, C_LNS0, C_SXM0, C_FOC0),
                 (P1, T1, 64, C_LNS1, C_SXM1, C_FOC1)]

        # ---- sigmoid / BCE / focal per row-tile ----
        ptiles = []
        for (Pt, Tt, np_, c_lns, c_sxm, c_foc) in parts:
            e = pool.tile([128, W], F32, tag=f"e{np_}")
            nc.scalar.activation(e[:np_], Pt[:np_], AF.Exp, scale=-1.0)
            s = pool.tile([128, W], F32, tag=f"s{np_}")
            nc.vector.tensor_scalar(s[:np_], e[:np_], 1.0, None, op0=OP.add)
            lns = pool.tile([128, W], F32, tag=f"lns{np_}")
            nc.scalar.activation(lns[:np_], s[:np_], AF.Ln,
                                 accum_out=OUT[:np_, c_lns:c_lns + 1])
            p = pool.tile([128, W], F32, tag=f"p{np_}")
            nc.scalar.activation(p[:np_], lns[:np_], AF.Exp, scale=-1.0)
            ptiles.append(p)
            # sum x*(t-1)  (BCE linear term)
            sscr = pool.tile([128, W], F32, tag=f"sscr{np_}")
            nc.vector.scalar_tensor_tensor(
                out=sscr[:np_], in0=Tt[:np_], scalar=1.0, in1=Pt[:np_],
                op0=OP.subtract, op1=OP.mult,
                accum_out=OUT[:np_, c_sxm:c_sxm + 1])
            # focal
            pc = pool.tile([128, W], F32, tag=f"pc{np_}")
            nc.vector.tensor_scalar(pc[:np_], p[:np_], EPS, 1.0 - EPS,
                                    op0=OP.max, op1=OP.min)
            u = pool.tile([128, W], F32, tag=f"u{np_}")
            nc.vector.tensor_tensor(u[:np_], Tt[:np_], pc[:np_], op=OP.subtract)
            u2 = pool.tile([128, W], F32, tag=f"u2{np_}")
            nc.vector.tensor_tensor(u2[:np_], u[:np_], u[:np_], op=OP.mult)
            au = pool.tile([128, W], F32, tag=f"au{np_}")
            nc.scalar.activation(au[:np_], u[:np_], AF.Abs)
            lnpt = pool.tile([128, W], F32, tag=f"lnpt{np_}")
            nc.scalar.activation(lnpt[:np_], au[:np_], AF.Ln,
                                 scale=-1.0, bias=1.0)
            at = pool.tile([128, W], F32, tag=f"at{np_}")
            nc.vector.tensor_scalar(at[:np_], Tt[:np_], -0.5, 0.75,
                                    op0=OP.mult, op1=OP.add)
            m1 = pool.tile([128, W], F32, tag=f"m1{np_}")
            nc.vector.tensor_tensor(m1[:np_], at[:np_], u2[:np_], op=OP.mult)
            fscr = pool.tile([128, W], F32, tag=f"fscr{np_}")
            nc.vector.scalar_tensor_tensor(
                out=fscr[:np_], in0=m1[:np_], scalar=1.0, in1=lnpt[:np_],
                op0=OP.mult, op1=OP.mult,
                accum_out=OUT[:np_, c_foc:c_foc + 1])
        p0, p1 = ptiles

        # ---- EDT masks packed [fg | bg] along free, padded with EBIG ----
        M0 = pool.tile([128, 2 * PADF], EDT_DT)
        nc.vector.memset(M0[:], EBIG)
        M1 = pool.tile([128, 2 * PADF], EDT_DT)
        nc.vector.memset(M1[:], EBIG)
        for (Mt, Tt, np_) in ((M0, T0, 128), (M1, T1, 64)):
            nc.vector.tensor_scalar(Mt[:np_, R:R + W], Tt[:np_], -EBIG, EBIG,
                                    op0=OP.mult, op1=OP.add)
            nc.vector.tensor_scalar(Mt[:np_, PADF + R:PADF + R + W], Tt[:np_],
                                    EBIG, None, op0=OP.mult)

        def winpass2_small(src, dst, np_, nseg, sseg, dseg):
            """dst[p, seg*dseg + x] = min_{|d|<=R} (src[p, seg*sseg + x+R+d]
            + d^2), seg in [0,nseg): batched min-plus via parabola symmetry
            (R pair-mins + fused add-min chain)."""
            sb = src[:]
            db = dst[:]

            def sap(off):
                return bass.AP(tensor=sb.tensor, offset=sb.offset + off,
                               ap=[[sb.ap[0][0], np_], [sseg, nseg], [1, W]])

            def dap(t):
                b = t[:]
                return bass.AP(tensor=b.tensor, offset=b.offset,
                               ap=[[b.ap[0][0], np_], [dseg, nseg], [1, W]])

            mds = []
            for dd in range(1, R + 1):
                md = pool.tile([128, nseg * W], EDT_DT, tag=f"md{dd}")
                nc.vector.tensor_tensor(dap(md), sap(R - dd), sap(R + dd),
                                        op=OP.min)
                mds.append(md)
            acc = sap(R)
            for dd in range(1, R + 1):
                if dd == R:
                    o = bass.AP(tensor=db.tensor, offset=db.offset,
                                ap=[[db.ap[0][0], np_], [dseg, nseg], [1, W]])
                else:
                    sc = pool.tile([128, nseg * W], EDT_DT, tag=f"sc{dd}")
                    o = dap(sc)
                nc.vector.scalar_tensor_tensor(
                    out=o, in0=dap(mds[dd - 1]), scalar=float(dd * dd),
                    in1=acc, op0=OP.add, op1=OP.min)
                acc = o

        def winpass2_big(src, dst, np_, nseg, sseg, dseg):
            for seg in range(nseg):
                for xb in range(0, W, XB):
                    xn = min(XB, W - xb)
                    b = src[:]
                    src_ap = bass.AP(tensor=b.tensor,
                                     offset=b.offset + seg * sseg + xb,
                                     ap=[[b.ap[0][0], np_], [1, xn], [1, K]])
                    qb = qt[:]
                    q_ap = bass.AP(tensor=qb.tensor, offset=qb.offset,
                                   ap=[[K, np_], [0, xn], [1, K]])
                    wb = wtmp[:]
                    tmp_w = bass.AP(tensor=wb.tensor, offset=wb.offset,
                                    ap=[[XB * K, np_], [K, xn], [1, K]])
                    nc.vector.tensor_tensor(tmp_w, src_ap, q_ap, op=OP.add)
                    nc.vector.tensor_reduce(
                        dst[:np_, seg * dseg + xb:seg * dseg + xb + xn],
                        tmp_w, axis=AX.X, op=OP.min)

        if small:
            winpass2 = winpass2_small
        else:
            wtmp = pool.tile([128, XB * K], F32)
            winpass2 = winpass2_big

        # ---- stage 1: row-distance^2, packed [fg|bg] ----
        C0 = pool.tile([128, 2 * W], EDT_DT)
        C1 = pool.tile([128, 2 * W], EDT_DT)
        winpass2(M0, C0, 128, 2, PADF, W)
        winpass2(M1, C1, 64, 2, PADF, W)

        # ---- transpose colmin into padded packed column-major tiles ----
        CT0 = pool.tile([128, 2 * PADF], EDT_DT)
        CT1 = pool.tile([128, 2 * PADF], EDT_DT)
        nc.vector.memset(CT0[:], EBIG)
        nc.vector.memset(CT1[:], EBIG)
        tidc = idtb if small else idt
        for (srcT, rn, ro) in ((C0, 128, 0), (C1, 64, 128)):
            for seg, soff in ((0, 0), (1, W)):       # fg, bg
                for (cb, cn, dstt) in ((0, 128, CT0), (128, 64, CT1)):
                    ps = psp.tile([128, 128], EDT_DT, tag="trps")
                    nc.tensor.transpose(
                        ps[:cn, :rn], srcT[:rn, soff + cb:soff + cb + cn],
                        tidc[:rn, :rn])
                    nc.vector.tensor_copy(
                        dstt[:cn, seg * PADF + R + ro:seg * PADF + R + ro + rn],
                        ps[:cn, :rn])

        # ---- stage 2: full distance^2 packed [fg|bg], [x_p, y] ----
        D20 = pool.tile([128, 2 * W], F32)
        D21 = pool.tile([128, 2 * W], F32)
        winpass2(CT0, D20, 128, 2, PADF, W)
        winpass2(CT1, D21, 64, 2, PADF, W)

        # ---- transpose p -> p_T ----
        PT0 = pool.tile([128, W], F32)
        PT1 = pool.tile([128, W], F32)
        for (srcT, rn, ro) in ((p0, 128, 0), (p1, 64, 128)):
            for (cb, cn, dstt) in ((0, 128, PT0), (128, 64, PT1)):
                ps2 = psp.tile([128, 128], F32, tag="trpsf")
                nc.tensor.transpose(ps2[:cn, :rn], srcT[:rn, cb:cb + cn],
                                    idt[:rn, :rn])
                nc.scalar.copy(dstt[:cn, ro:ro + rn], ps2[:cn, :rn])

        # ---- boundary loss: sum p * sqrt(d2) * exp(-sqrt(d2)/10) ----
        for i, (D2, PT, np_) in enumerate(((D20, PT0, 128), (D21, PT1, 64))):
            dc = pool.tile([128, 2 * W], F32, tag=f"dc{i}")
            nc.vector.tensor_scalar(dc[:np_], D2[:np_], 1e-30, MAXD2,
                                    op0=OP.max, op1=OP.min)
            nc.vector.tensor_reduce(OUT[:np_, C_CERT[i]:C_CERT[i] + 1],
                                    dc[:np_], axis=AX.X, op=OP.max)
            lnd = pool.tile([128, 2 * W], F32, tag=f"lnd{i}")
            nc.scalar.activation(lnd[:np_], dc[:np_], AF.Ln)
            phi = pool.tile([128, 2 * W], F32, tag=f"phi{i}")
            nc.scalar.activation(phi[:np_], lnd[:np_], AF.Exp, scale=0.5)
            wb = pool.tile([128, 2 * W], F32, tag=f"wb{i}")
            nc.scalar.activation(wb[:np_], phi[:np_], AF.Exp, scale=-0.1)
            fw = pool.tile([128, 2 * W], F32, tag=f"fw{i}")
            nc.vector.tensor_tensor(fw[:np_], phi[:np_], wb[:np_], op=OP.mult)
            ptb = PT[:]
            pt2 = bass.AP(tensor=ptb.tensor, offset=ptb.offset,
                          ap=[[ptb.ap[0][0], np_], [0, 2], [1, W]])
            fwb = fw[:]
            fw2 = bass.AP(tensor=fwb.tensor, offset=fwb.offset,
                          ap=[[fwb.ap[0][0], np_], [W, 2], [1, W]])
            bscr = pool.tile([128, 2 * W], F32, tag=f"bscr{i}")
            bsb = bscr[:]
            bs2 = bass.AP(tensor=bsb.tensor, offset=bsb.offset,
                          ap=[[bsb.ap[0][0], np_], [W, 2], [1, W]])
            nc.vector.scalar_tensor_tensor(
                out=bs2, in0=fw2, scalar=1.0, in1=pt2,
                op0=OP.mult, op1=OP.mult,
                accum_out=OUT[:np_, C_BND[i]:C_BND[i] + 1])

        # ---- SSIM ----
        X2_0 = pool.tile([128, W], F32)
        nc.vector.tensor_tensor(X2_0[:], p0[:], p0[:], op=OP.mult)
        X2_1 = pool.tile([128, W], F32)
        nc.vector.tensor_tensor(X2_1[:64], p1[:64], p1[:64], op=OP.mult)
        XY_0 = pool.tile([128, W], F32)
        nc.vector.tensor_tensor(XY_0[:], p0[:], T0[:], op=OP.mult)
        XY_1 = pool.tile([128, W], F32)
        nc.vector.tensor_tensor(XY_1[:64], p1[:64], T1[:64], op=OP.mult)

        maps = [("mux", p0, p1), ("muy", T0, T1),
                ("mxx", X2_0, X2_1), ("mxy", XY_0, XY_1)]
        # conv1 transposed out: C1T[x, ro] = sum_r M[r, x] G[r, ro]
        C1T_x0 = pool.tile([128, 4 * CONV], F32)
        C1T_x1 = pool.tile([128, 4 * CONV], F32)
        for mi, (name, M0, M1) in enumerate(maps):
            for (xc, xn, slab) in ((0, 128, C1T_x0), (128, 64, C1T_x1)):
                c1 = psc.tile([128, CONV], F32, tag="c1t")
                nc.tensor.matmul(c1[:xn, :], M0[:, xc:xc + xn], g0[:],
                                 start=True, stop=False)
                nc.tensor.matmul(c1[:xn, :], M1[:64, xc:xc + xn], g1[:64],
                                 start=False, stop=True)
                nc.scalar.copy(slab[:xn, mi * CONV:(mi + 1) * CONV], c1[:xn, :])
        # conv2: C2[xo, map*CONV+ro] = sum_x G[x, xo] C1T[x, map*CONV+ro]
        conv = {}
        c2s_a = pool.tile([128, 4 * CONV], F32)
        c2s_b = pool.tile([128, 4 * CONV], F32)
        for (fc, fn) in ((0, 2 * CONV), (2 * CONV, 2 * CONV)):
            for (oc, on, dst) in ((0, 128, c2s_a), (128, 54, c2s_b)):
                c2 = psc.tile([128, 2 * CONV], F32, tag="c2t")
                nc.tensor.matmul(c2[:on, :fn], g0[:, oc:oc + on],
                                 C1T_x0[:, fc:fc + fn], start=True, stop=False)
                nc.tensor.matmul(c2[:on, :fn], g1[:64, oc:oc + on],
                                 C1T_x1[:64, fc:fc + fn], start=False, stop=True)
                nc.scalar.copy(dst[:on, fc:fc + fn], c2[:on, :fn])
        for mi, (name, _, _) in enumerate(maps):
            conv[name] = (c2s_a[:, mi * CONV:(mi + 1) * CONV],
                          c2s_b[:, mi * CONV:(mi + 1) * CONV])

        C1c = 0.01 ** 2
        C2c = 0.03 ** 2
        for half, np_, ccol in ((0, 128, C_SSA), (1, 54, C_SSB)):
            mux = conv["mux"][half]
            muy = conv["muy"][half]
            mxx = conv["mxx"][half]
            mxy = conv["mxy"][half]
            A = pool.tile([128, CONV], F32, tag=f"ssA{half}")
            nc.vector.tensor_tensor(A[:np_], mux[:np_], muy[:np_], op=OP.mult)
            B = pool.tile([128, CONV], F32, tag=f"ssB{half}")
            nc.vector.tensor_tensor(B[:np_], mux[:np_], mux[:np_], op=OP.mult)
            Cc = pool.tile([128, CONV], F32, tag=f"ssC{half}")
            nc.vector.tensor_tensor(Cc[:np_], muy[:np_], muy[:np_], op=OP.mult)
            bc = pool.tile([128, CONV], F32, tag=f"ssbc{half}")
            nc.vector.tensor_tensor(bc[:np_], B[:np_], Cc[:np_], op=OP.add)
            den1 = pool.tile([128, CONV], F32, tag=f"ssd1{half}")
            nc.vector.tensor_scalar(den1[:np_], bc[:np_], C1c, None, op0=OP.add)
            t1 = pool.tile([128, CONV], F32, tag=f"sst1{half}")
            nc.vector.tensor_tensor(t1[:np_], mxx[:np_], muy[:np_], op=OP.add)
            t2 = pool.tile([128, CONV], F32, tag=f"sst2{half}")
            nc.vector.tensor_tensor(t2[:np_], t1[:np_], bc[:np_], op=OP.subtract)
            den2 = pool.tile([128, CONV], F32, tag=f"ssd2{half}")
            nc.vector.tensor_scalar(den2[:np_], t2[:np_], C2c, None, op0=OP.add)
            den = pool.tile([128, CONV], F32, tag=f"ssden{half}")
            nc.vector.tensor_tensor(den[:np_], den1[:np_], den2[:np_], op=OP.mult)
            lnden = pool.tile([128, CONV], F32, tag=f"sslnd{half}")
            nc.scalar.activation(lnden[:np_], den[:np_], AF.Ln)
            rden = pool.tile([128, CONV], F32, tag=f"ssrd{half}")
            nc.scalar.activation(rden[:np_], lnden[:np_], AF.Exp, scale=-1.0)
            n1 = pool.tile([128, CONV], F32, tag=f"ssn1{half}")
            nc.vector.tensor_scalar(n1[:np_], A[:np_], 2.0, C1c,
                                    op0=OP.mult, op1=OP.add)
            t3 = pool.tile([128, CONV], F32, tag=f"sst3{half}")
            nc.vector.tensor_tensor(t3[:np_], mxy[:np_], A[:np_], op=OP.subtract)
            n2 = pool.tile([128, CONV], F32, tag=f"ssn2{half}")
            nc.vector.tensor_scalar(n2[:np_], t3[:np_], 2.0, C2c,
                                    op0=OP.mult, op1=OP.add)
            num = pool.tile([128, CONV], F32, tag=f"ssnum{half}")
            nc.vector.tensor_tensor(num[:np_], n1[:np_], n2[:np_], op=OP.mult)
            sscr2 = pool.tile([128, CONV], F32, tag=f"ssfin{half}")
            nc.vector.scalar_tensor_tensor(
                out=sscr2[:np_], in0=num[:np_], scalar=1.0, in1=rden[:np_],
                op0=OP.mult, op1=OP.mult,
                accum_out=OUT[:np_, ccol:ccol + 1])

        nc.gpsimd.dma_start(outd[:], OUT[:])
    nc.finalize()
    return nc


def _gauss_mat():
    ks, sigma = 11, 1.5
    c = (ks - 1) / 2.0
    g = np.exp(-((np.arange(ks) - c) ** 2) / (2.0 * sigma ** 2))
    g = (g / g.sum()).astype(np.float32)
    G = np.zeros((H, CONV), np.float32)
    for o in range(CONV):
        G[o:o + ks, o] = g
    return G


def _bf16_np():
    import ml_dtypes
    return np.dtype(ml_dtypes.bfloat16)


_CACHE = {}
_TRACE = {"enabled": False, "last": None}


def _get_kernel(R, XB):
    key = (R, XB)
    if key not in _CACHE:
        _CACHE[key] = build_kernel(R, XB)
    return _CACHE[key]


def _run(nc, R, pred_masks, target_masks):
    K = 2 * R + 1
    G = _gauss_mat()
    ident = np.eye(128, dtype=np.float32)
    in_maps = []
    for c in range(8):
        m = {
            "pred": np.ascontiguousarray(pred_masks[c, 0]).astype(np.float32),
            "targ": np.ascontiguousarray(target_masks[c, 0]).astype(np.float32),
            "gmat": G, "ident": ident,
            "identb": ident.astype(_bf16_np()),
        }
        if R > 8:
            m["qwin"] = np.broadcast_to(
                ((np.arange(K, dtype=np.float32) - R) ** 2)[None, :],
                (128, K)).copy()
        in_maps.append(m)
    res = run_bass_kernel_spmd(nc, in_maps, core_ids=list(range(8)),
                               trace=_TRACE["enabled"])
    _TRACE["last"] = res
    return [r["out"].astype(np.float64) for r in res.results]


def _combine(outs):
    N = 8 * H * W
    lns = sxm = foc = bnd = ssim = 0.0
    maxd2 = 0.0
    for o in outs:
        lns += o[:, C_LNS0].sum() + o[:, C_LNS1].sum()
        sxm += o[:, C_SXM0].sum() + o[:, C_SXM1].sum()
        foc += o[:, C_FOC0].sum() + o[:, C_FOC1].sum()
        for c in C_BND:
            bnd += o[:, c].sum()
        ssim += o[:, C_SSA].sum() + o[:, C_SSB].sum()
        for c in C_CERT:
            maxd2 = max(maxd2, o[:, c].max())
    bce = (lns - sxm) / N
    boundary = bnd / N
    focal = -foc / N
    ssim_v = ssim / (8 * CONV * CONV)
    return bce + boundary + focal + ssim_v, maxd2


def kernel(pred_masks, target_masks):
    R = 3
    nc = _get_kernel(R, 192)
    outs = _run(nc, R, pred_masks, target_masks)
    total, maxd2 = _combine(outs)
    if not (maxd2 < (R + 1) ** 2):
        Rf = 191
        ncf = _get_kernel(Rf, 16)
        outs = _run(ncf, Rf, pred_masks, target_masks)
        total, _ = _combine(outs)
    return np.float32(total)


# revision 12
# speedup vs baseline: 1.1581x; 1.1581x over previous
"""Bass/Trainium2 kernel for nn_MetaLoss_32255204393276.

Loss = BCE(sigmoid(pred), t) + boundary(EDT) + focal + SSIM over 8 pairs of
[1,192,192] masks; data-parallel over 8 NeuronCores (one image per core).
Each core returns [128, NCOL] partial sums; the host combines them.

EDT strategy: exact Euclidean distance transform via two min-plus passes
(rows then columns). For small window radius R the parabola (j-R)^2 is
symmetric, so the windowed min-plus is computed as R pair-mins of shifted
slices plus a fused (add-const, min) chain - exact since all finite values
are small integers (representable in bf16). Windowed results equal the true
EDT whenever max d^2 < (R+1)^2, certified on device; on failure (never for
the Bernoulli masks this problem generates) a full-window R=191 kernel
reruns, which is unconditionally exact including the empty-mask clamp.
"""

import numpy as np
from contextlib import ExitStack

import concourse.bass as bass
import concourse.bacc as bacc
import concourse.hw_specs as hw_specs
import concourse.tile as tile
from concourse.vector_clock import ScopedClock
from concourse import mybir
from concourse.bass_utils import run_bass_kernel_spmd

F32 = mybir.dt.float32
BF16 = mybir.dt.bfloat16
AF = mybir.ActivationFunctionType
OP = mybir.AluOpType
AX = mybir.AxisListType

H = W = 192
CONV = 182          # 192 - 11 + 1
BIG = 1e10
MAXD2 = float((H - 1) ** 2 + (W - 1) ** 2)  # 72962
EPS = 1e-6

# output columns
C_LNS0, C_LNS1 = 0, 1        # sum ln(1+e^-x)
C_SXM0, C_SXM1 = 2, 3        # sum x*(t-1)
C_FOC0, C_FOC1 = 4, 5        # sum alpha_t*(1-pt)^2*ln(pt)
C_BND = (6, 7, 8, 9)         # sum p*f(d2) per (F0,B0,F1,B1)
C_CERT = (10, 11, 12, 13)    # max clamped d2 per tile
C_SSA, C_SSB = 14, 15        # ssim ratio sums
NCOL = 16

_orig_get_tables = hw_specs.get_activation_tables


def _one_set_tables(module_arch):
    # all activation funcs we use (exp/ln/copy/abs/identity) live in this one
    # set; restricting the choice stops the compiler from thrashing tables
    tabs = _orig_get_tables(module_arch)
    name = "natural_log_exp_and_others"
    return {k: (v if k == name else set()) for k, v in tabs.items()}


def _light_drain_and_barrier(self, tick_clock, wait_clock):
    # Lighter kernel tail than TileContext._drain_and_barrier: the semaphore
    # clears all run on GPSIMD, so one gpsimd drain that waits on the global
    # tick clock (= every scheduled instruction completed, including every
    # instruction carrying a sem wait) makes the clears race-free without two
    # all-engine barriers.
    drain_inst = self.nc.gpsimd.drain()
    wait_clock.add_sem_waits(
        drain_inst.ins, ScopedClock({None: tick_clock.global_clock}))
    popped = self.nc._tile_sem_poison_stack.pop()
    assert popped is self._sem_poison
    self.nc.clear_and_free_semaphores(list(self.sems.allocated().values()))


def build_kernel(R=3, XB=192):
    K = 2 * R + 1
    PADF = W + 2 * R
    small = R <= 8
    EDT_DT = BF16 if small else F32
    EBIG = 512.0 if small else BIG

    bacc.get_activation_tables = _one_set_tables
    tile.TileContext._drain_and_barrier = _light_drain_and_barrier
    nc = bacc.Bacc(None, target_bir_lowering=False)
    pred = nc.dram_tensor("pred", [H, W], F32, kind="ExternalInput")
    targ = nc.dram_tensor("targ", [H, W], F32, kind="ExternalInput")
    if not small:
        qwin = nc.dram_tensor("qwin", [128, K], F32, kind="ExternalInput")
    gmat = nc.dram_tensor("gmat", [H, CONV], F32, kind="ExternalInput")
    ident = nc.dram_tensor("ident", [128, 128], F32, kind="ExternalInput")
    identb = nc.dram_tensor("identb", [128, 128], BF16, kind="ExternalInput")
    outd = nc.dram_tensor("out", [128, NCOL], F32, kind="ExternalOutput")

    with tile.TileContext(nc) as tc, ExitStack() as ctx:
        pool = ctx.enter_context(tc.tile_pool(name="main", bufs=1))
        psp = ctx.enter_context(tc.tile_pool(name="ps", bufs=3, space="PSUM"))
        psc = ctx.enter_context(tc.tile_pool(name="psc", bufs=1, space="PSUM"))

        OUT = pool.tile([128, NCOL], F32)
        nc.vector.memset(OUT[:], 0.0)

        # ---- loads ----
        P0 = pool.tile([128, W], F32)
        nc.sync.dma_start(P0[:], pred[0:128, :])
        P1 = pool.tile([128, W], F32)
        nc.sync.dma_start(P1[:64, :], pred[128:192, :])
        T0 = pool.tile([128, W], F32)
        nc.sync.dma_start(T0[:], targ[0:128, :])
        T1 = pool.tile([128, W], F32)
        nc.sync.dma_start(T1[:64, :], targ[128:192, :])
        if not small:
            qt = pool.tile([128, K], F32)
            nc.gpsimd.dma_start(qt[:], qwin[:])
        g0 = pool.tile([128, CONV], F32)
        nc.gpsimd.dma_start(g0[:], gmat[0:128, :])
        g1 = pool.tile([128, CONV], F32)
        nc.gpsimd.dma_start(g1[:64, :], gmat[128:192, :])
        idt = pool.tile([128, 128], F32)
        nc.gpsimd.dma_start(idt[:], ident[:])
        idtb = pool.tile([128, 128], BF16)
        nc.gpsimd.dma_start(idtb[:], identb[:])

        parts = [(P0, T0, 128, C_LNS0, C_SXM0, C_FOC0),
                 (P1, T1, 64, C_LNS1, C_SXM1, C_FOC1)]

        # ---- sigmoid / BCE / focal per row-tile ----
        ptiles = []
        for (Pt, Tt, np_, c_lns, c_sxm, c_foc) in parts:
            e = pool.tile([128, W], F32, tag=f"e{np_}")
            nc.scalar.activation(e[:np_], Pt[:np_], AF.Exp, scale=-1.0)
            s = pool.tile([128, W], F32, tag=f"s{np_}")
            nc.vector.tensor_scalar(s[:np_], e[:np_], 1.0, None, op0=OP.add)
            lns = pool.tile([128, W], F32, tag=f"lns{np_}")
            nc.scalar.activation(lns[:np_], s[:np_], AF.Ln,
                                 accum_out=OUT[:np_, c_lns:c_lns + 1])
            p = pool.tile([128, W], F32, tag=f"p{np_}")
            nc.scalar.activation(p[:np_], lns[:np_], AF.Exp, scale=-1.0)
            ptiles.append(p)
            # sum x*(t-1)  (BCE linear term)
            sscr = pool.tile([128, W], F32, tag=f"sscr{np_}")
            nc.vector.scalar_tensor_tensor(
                out=sscr[:np_], in0=Tt[:np_], scalar=1.0, in1=Pt[:np_],
                op0=OP.subtract, op1=OP.mult,
                accum_out=OUT[:np_, c_sxm:c_sxm + 1])
            # focal
            pc = pool.tile([128, W], F32, tag=f"pc{np_}")
            nc.vector.tensor_scalar(pc[:np_], p[:np_], EPS, 1.0 - EPS,
                                    op0=OP.max, op1=OP.min)
            u = pool.tile([128, W], F32, tag=f"u{np_}")
            nc.vector.tensor_tensor(u[:np_], Tt[:np_], pc[:np_], op=OP.subtract)
            u2 = pool.tile([128, W], F32, tag=f"u2{np_}")
            nc.vector.tensor_tensor(u2[:np_], u[:np_], u[:np_], op=OP.mult)
            au = pool.tile([128, W], F32, tag=f"au{np_}")
            nc.scalar.activation(au[:np_], u[:np_], AF.Abs)
            lnpt = pool.tile([128, W], F32, tag=f"lnpt{np_}")
            nc.scalar.activation(lnpt[:np_], au[:np_], AF.Ln,
                                 scale=-1.0, bias=1.0)
            at = pool.tile([128, W], F32, tag=f"at{np_}")
            nc.vector.tensor_scalar(at[:np_], Tt[:np_], -0.5, 0.75,
                                    op0=OP.mult, op1=OP.add)
            m1 = pool.tile([128, W], F32, tag=f"m1{np_}")
            nc.vector.tensor_tensor(m1[:np_], at[:np_], u2[:np_], op=OP.mult)
            fscr = pool.tile([128, W], F32, tag=f"fscr{np_}")
            nc.vector.scalar_tensor_tensor(
                out=fscr[:np_], in0=m1[:np_], scalar=1.0, in1=lnpt[:np_],
                op0=OP.mult, op1=OP.mult,
                accum_out=OUT[:np_, c_foc:c_foc + 1])
        p0, p1 = ptiles

        # ---- EDT masks (stage 1 input), padded with EBIG ----
        def mk_mask(name, Tt, np_, neg):
            m = pool.tile([128, PADF], EDT_DT, tag=name)
            nc.vector.memset(m[:], EBIG)
            if neg:
                nc.vector.tensor_scalar(m[:np_, R:R + W], Tt[:np_], -EBIG,
                                        EBIG, op0=OP.mult, op1=OP.add)
            else:
                nc.vector.tensor_scalar(m[:np_, R:R + W], Tt[:np_], EBIG,
                                        None, op0=OP.mult)
            return m

        MF0 = mk_mask("MF0", T0, 128, True)
        MF1 = mk_mask("MF1", T1, 64, True)
        MB0 = mk_mask("MB0", T0, 128, False)
        MB1 = mk_mask("MB1", T1, 64, False)

        if small:
            def winpass(src, dst, np_):
                """dst[p, x] = min_{|d|<=R} (src[p, x+R+d] + d^2) using the
                parabola's symmetry: R pair-mins + fused add-min chain."""
                mds = []
                for dd in range(1, R + 1):
                    md = pool.tile([128, W], EDT_DT, tag=f"md{dd}")
                    nc.vector.tensor_tensor(
                        md[:np_], src[:np_, R - dd:R - dd + W],
                        src[:np_, R + dd:R + dd + W], op=OP.min)
                    mds.append(md)
                acc = src[:np_, R:R + W]
                for dd in range(1, R + 1):
                    if dd == R:
                        o = dst[:np_, 0:W]
                    else:
                        sc = pool.tile([128, W], EDT_DT, tag=f"sc{dd}")
                        o = sc[:np_]
                    nc.vector.scalar_tensor_tensor(
                        out=o, in0=mds[dd - 1][:np_], scalar=float(dd * dd),
                        in1=acc, op0=OP.add, op1=OP.min)
                    acc = o
        else:
            wtmp = pool.tile([128, XB * K], F32)

            def winpass(src, dst, np_):
                for xb in range(0, W, XB):
                    xn = min(XB, W - xb)
                    b = src[:]
                    src_ap = bass.AP(tensor=b.tensor, offset=b.offset + xb,
                                     ap=[[PADF, np_], [1, xn], [1, K]])
                    qb = qt[:]
                    q_ap = bass.AP(tensor=qb.tensor, offset=qb.offset,
                                   ap=[[K, np_], [0, xn], [1, K]])
                    wb = wtmp[:]
                    tmp_w = bass.AP(tensor=wb.tensor, offset=wb.offset,
                                    ap=[[XB * K, np_], [K, xn], [1, K]])
                    nc.vector.tensor_tensor(tmp_w, src_ap, q_ap, op=OP.add)
                    nc.vector.tensor_reduce(
                        dst[:np_, xb:xb + xn], tmp_w, axis=AX.X, op=OP.min)

        # ---- stage 1: row-distance^2 ----
        CF0 = pool.tile([128, W], EDT_DT)
        CF1 = pool.tile([128, W], EDT_DT)
        CB0 = pool.tile([128, W], EDT_DT)
        CB1 = pool.tile([128, W], EDT_DT)
        winpass(MF0, CF0, 128)
        winpass(MF1, CF1, 64)
        winpass(MB0, CB0, 128)
        winpass(MB1, CB1, 64)

        # ---- transpose colmin into padded column-major tiles ----
        def transpose_192(src0, src1, dst0, dst1, coff, dt_, copy_eng):
            tid = idtb if dt_ == BF16 else idt
            for (srcc, rn, ro) in ((src0, 128, 0), (src1, 64, 128)):
                for (cb, cn, dstt) in ((0, 128, dst0), (128, 64, dst1)):
                    ps = psp.tile([128, 128], dt_, tag=f"trps{dt_}")
                    nc.tensor.transpose(ps[:cn, :rn], srcc[:rn, cb:cb + cn],
                                        tid[:rn, :rn])
                    copy_eng(dstt[:cn, coff + ro:coff + ro + rn], ps[:cn, :rn])

        CTF0 = pool.tile([128, PADF], EDT_DT)
        CTF1 = pool.tile([128, PADF], EDT_DT)
        CTB0 = pool.tile([128, PADF], EDT_DT)
        CTB1 = pool.tile([128, PADF], EDT_DT)
        for t in (CTF0, CTF1, CTB0, CTB1):
            nc.vector.memset(t[:], EBIG)
        transpose_192(CF0, CF1, CTF0, CTF1, R, EDT_DT, nc.vector.tensor_copy)
        transpose_192(CB0, CB1, CTB0, CTB1, R, EDT_DT, nc.vector.tensor_copy)

        # ---- stage 2: full distance^2, [x_p, y] ----
        D2F0 = pool.tile([128, W], EDT_DT)
        D2F1 = pool.tile([128, W], EDT_DT)
        D2B0 = pool.tile([128, W], EDT_DT)
        D2B1 = pool.tile([128, W], EDT_DT)
        winpass(CTF0, D2F0, 128)
        winpass(CTF1, D2F1, 64)
        winpass(CTB0, D2B0, 128)
        winpass(CTB1, D2B1, 64)

        # ---- transpose p -> p_T ----
        PT0 = pool.tile([128, W], F32)
        PT1 = pool.tile([128, W], F32)
        transpose_192(p0, p1, PT0, PT1, 0, F32, nc.scalar.copy)

        # ---- boundary loss: sum p * sqrt(d2) * exp(-sqrt(d2)/10) ----
        for i, (D2, PT, np_) in enumerate(((D2F0, PT0, 128), (D2B0, PT0, 128),
                                           (D2F1, PT1, 64), (D2B1, PT1, 64))):
            dc = pool.tile([128, W], F32, tag=f"dc{i}")
            nc.vector.tensor_scalar(dc[:np_], D2[:np_], 1e-30, MAXD2,
                                    op0=OP.max, op1=OP.min)
            nc.vector.tensor_reduce(OUT[:np_, C_CERT[i]:C_CERT[i] + 1],
                                    dc[:np_], axis=AX.X, op=OP.max)
            lnd = pool.tile([128, W], F32, tag=f"lnd{i}")
            nc.scalar.activation(lnd[:np_], dc[:np_], AF.Ln)
            phi = pool.tile([128, W], F32, tag=f"phi{i}")
            nc.scalar.activation(phi[:np_], lnd[:np_], AF.Exp, scale=0.5)
            wb = pool.tile([128, W], F32, tag=f"wb{i}")
            nc.scalar.activation(wb[:np_], phi[:np_], AF.Exp, scale=-0.1)
            fw = pool.tile([128, W], F32, tag=f"fw{i}")
            nc.vector.tensor_tensor(fw[:np_], phi[:np_], wb[:np_], op=OP.mult)
            bscr = pool.tile([128, W], F32, tag=f"bscr{i}")
            nc.vector.scalar_tensor_tensor(
                out=bscr[:np_], in0=fw[:np_], scalar=1.0, in1=PT[:np_],
                op0=OP.mult, op1=OP.mult,
                accum_out=OUT[:np_, C_BND[i]:C_BND[i] + 1])

        # ---- SSIM ----
        X2_0 = pool.tile([128, W], F32)
        nc.vector.tensor_tensor(X2_0[:], p0[:], p0[:], op=OP.mult)
        X2_1 = pool.tile([128, W], F32)
        nc.vector.tensor_tensor(X2_1[:64], p1[:64], p1[:64], op=OP.mult)
        XY_0 = pool.tile([128, W], F32)
        nc.vector.tensor_tensor(XY_0[:], p0[:], T0[:], op=OP.mult)
        XY_1 = pool.tile([128, W], F32)
        nc.vector.tensor_tensor(XY_1[:64], p1[:64], T1[:64], op=OP.mult)

        maps = [("mux", p0, p1), ("muy", T0, T1),
                ("mxx", X2_0, X2_1), ("mxy", XY_0, XY_1)]
        # conv1 transposed out: C1T[x, ro] = sum_r M[r, x] G[r, ro]
        C1T_x0 = pool.tile([128, 4 * CONV], F32)
        C1T_x1 = pool.tile([128, 4 * CONV], F32)
        for mi, (name, M0, M1) in enumerate(maps):
            for (xc, xn, slab) in ((0, 128, C1T_x0), (128, 64, C1T_x1)):
                c1 = psc.tile([128, CONV], F32, tag="c1t")
                nc.tensor.matmul(c1[:xn, :], M0[:, xc:xc + xn], g0[:],
                                 start=True, stop=False)
                nc.tensor.matmul(c1[:xn, :], M1[:64, xc:xc + xn], g1[:64],
                                 start=False, stop=True)
                nc.scalar.copy(slab[:xn, mi * CONV:(mi + 1) * CONV], c1[:xn, :])
        # conv2: C2[xo, map*CONV+ro] = sum_x G[x, xo] C1T[x, map*CONV+ro]
        conv = {}
        c2s_a = pool.tile([128, 4 * CONV], F32)
        c2s_b = pool.tile([128, 4 * CONV], F32)
        for (fc, fn) in ((0, 2 * CONV), (2 * CONV, 2 * CONV)):
            for (oc, on, dst) in ((0, 128, c2s_a), (128, 54, c2s_b)):
                c2 = psc.tile([128, 2 * CONV], F32, tag="c2t")
                nc.tensor.matmul(c2[:on, :fn], g0[:, oc:oc + on],
                                 C1T_x0[:, fc:fc + fn], start=True, stop=False)
                nc.tensor.matmul(c2[:on, :fn], g1[:64, oc:oc + on],
                                 C1T_x1[:64, fc:fc + fn], start=False, stop=True)
                nc.scalar.copy(dst[:on, fc:fc + fn], c2[:on, :fn])
        for mi, (name, _, _) in enumerate(maps):
            conv[name] = (c2s_a[:, mi * CONV:(mi + 1) * CONV],
                          c2s_b[:, mi * CONV:(mi + 1) * CONV])

        C1c = 0.01 ** 2
        C2c = 0.03 ** 2
        for half, np_, ccol in ((0, 128, C_SSA), (1, 54, C_SSB)):
            mux = conv["mux"][half]
            muy = conv["muy"][half]
            mxx = conv["mxx"][half]
            mxy = conv["mxy"][half]
            A = pool.tile([128, CONV], F32, tag=f"ssA{half}")
            nc.vector.tensor_tensor(A[:np_], mux[:np_], muy[:np_], op=OP.mult)
            B = pool.tile([128, CONV], F32, tag=f"ssB{half}")
            nc.vector.tensor_tensor(B[:np_], mux[:np_], mux[:np_], op=OP.mult)
            Cc = pool.tile([128, CONV], F32, tag=f"ssC{half}")
            nc.vector.tensor_tensor(Cc[:np_], muy[:np_], muy[:np_], op=OP.mult)
            bc = pool.tile([128, CONV], F32, tag=f"ssbc{half}")
            nc.vector.tensor_tensor(bc[:np_], B[:np_], Cc[:np_], op=OP.add)
            den1 = pool.tile([128, CONV], F32, tag=f"ssd1{half}")
            nc.vector.tensor_scalar(den1[:np_], bc[:np_], C1c, None, op0=OP.add)
            t1 = pool.tile([128, CONV], F32, tag=f"sst1{half}")
            nc.vector.tensor_tensor(t1[:np_], mxx[:np_], muy[:np_], op=OP.add)
            t2 = pool.tile([128, CONV], F32, tag=f"sst2{half}")
            nc.vector.tensor_tensor(t2[:np_], t1[:np_], bc[:np_], op=OP.subtract)
            den2 = pool.tile([128, CONV], F32, tag=f"ssd2{half}")
            nc.vector.tensor_scalar(den2[:np_], t2[:np_], C2c, None, op0=OP.add)
            den = pool.tile([128, CONV], F32, tag=f"ssden{half}")
            nc.vector.tensor_tensor(den[:np_], den1[:np_], den2[:np_], op=OP.mult)
            lnden = pool.tile([128, CONV], F32, tag=f"sslnd{half}")
            nc.scalar.activation(lnden[:np_], den[:np_], AF.Ln)
            rden = pool.tile([128, CONV], F32, tag=f"ssrd{half}")
            nc.scalar.activation(rden[:np_], lnden[:np_], AF.Exp, scale=-1.0)
            n1 = pool.tile([128, CONV], F32, tag=f"ssn1{half}")
            nc.vector.tensor_scalar(n1[:np_], A[:np_], 2.0, C1c,
                                    op0=OP.mult, op1=OP.add)
            t3 = pool.tile([128, CONV], F32, tag=f"sst3{half}")
            nc.vector.tensor_tensor(t3[:np_], mxy[:np_], A[:np_], op=OP.subtract)
            n2 = pool.tile([128, CONV], F32, tag=f"ssn2{half}")
            nc.vector.tensor_scalar(n2[:np_], t3[:np_], 2.0, C2c,
                                    op0=OP.mult, op1=OP.add)
            num = pool.tile([128, CONV], F32, tag=f"ssnum{half}")
            nc.vector.tensor_tensor(num[:np_], n1[:np_], n2[:np_], op=OP.mult)
            sscr2 = pool.tile([128, CONV], F32, tag=f"ssfin{half}")
            nc.vector.scalar_tensor_tensor(
                out=sscr2[:np_], in0=num[:np_], scalar=1.0, in1=rden[:np_],
                op0=OP.mult, op1=OP.mult,
                accum_out=OUT[:np_, ccol:ccol + 1])

        nc.gpsimd.dma_start(outd[:], OUT[:])
    nc.finalize()
    return nc


def _gauss_mat():
    ks, sigma = 11, 1.5
    c = (ks - 1) / 2.0
    g = np.exp(-((np.arange(ks) - c) ** 2) / (2.0 * sigma ** 2))
    g = (g / g.sum()).astype(np.float32)
    G = np.zeros((H, CONV), np.float32)
    for o in range(CONV):
        G[o:o + ks, o] = g
    return G


def _bf16_np():
    import ml_dtypes
    return np.dtype(ml_dtypes.bfloat16)


_CACHE = {}
_TRACE = {"enabled": False, "last": None}


def _get_kernel(R, XB):
    key = (R, XB)
    if key not in _CACHE:
        _CACHE[key] = build_kernel(R, XB)
    return _CACHE[key]


def _run(nc, R, pred_masks, target_masks):
    K = 2 * R + 1
    G = _gauss_mat()
    ident = np.eye(128, dtype=np.float32)
    in_maps = []
    for c in range(8):
        m = {
            "pred": np.ascontiguousarray(pred_masks[c, 0]).astype(np.float32),
            "targ": np.ascontiguousarray(target_masks[c, 0]).astype(np.float32),
            "gmat": G, "ident": ident,
            "identb": ident.astype(_bf16_np()),
        }
        if R > 8:
            m["qwin"] = np.broadcast_to(
                ((np.arange(K, dtype=np.float32) - R) ** 2)[None, :],
                (128, K)).copy()
        in_maps.append(m)
    res = run_bass_kernel_spmd(nc, in_maps, core_ids=list(range(8)),
                               trace=_TRACE["enabled"])
    _TRACE["last"] = res
    return [r["out"].astype(np.float64) for r in res.results]


def _combine(outs):
    N = 8 * H * W
    lns = sxm = foc = bnd = ssim = 0.0
    maxd2 = 0.0
    for o in outs:
        lns += o[:, C_LNS0].sum() + o[:, C_LNS1].sum()
        sxm += o[:, C_SXM0].sum() + o[:, C_SXM1].sum()
        foc += o[:, C_FOC0].sum() + o[:, C_FOC1].sum()
        for c in C_BND:
            bnd += o[:, c].sum()
        ssim += o[:, C_SSA].sum() + o[:, C_SSB].sum()
        for c in C_CERT:
            maxd2 = max(maxd2, o[:, c].max())
    bce = (lns - sxm) / N
    boundary = bnd / N
    focal = -foc / N
    ssim_v = ssim / (8 * CONV * CONV)
    return bce + boundary + focal + ssim_v, maxd2


def kernel(pred_masks, target_masks):
    R = 3
    nc = _get_kernel(R, 192)
    outs = _run(nc, R, pred_masks, target_masks)
    total, maxd2 = _combine(outs)
    if not (maxd2 < (R + 1) ** 2):
        Rf = 191
        ncf = _get_kernel(Rf, 16)
        outs = _run(ncf, Rf, pred_masks, target_masks)
        total, _ = _combine(outs)
    return np.float32(total)
